# revision 6
# baseline (speedup 1.0000x reference)
"""Trainium2 Bass kernel for nn_AutoAttention_Layer (sparse_attention).

Math (from the reference):
    W    = softmax(mss_weight, axis=1)                      # (3,3)
    qsum = sum_j q[b,j,:]                                   # (B,D)
    ksum_s[b,d] = sum_{l < len[b]} k[b,l,s*D+d]             # (B,3,D)
    s[r,b,d]    = (sum_s W[r,s]*ksum_s[b,d]) * qsum[b,d]
    out[b,0,r*D+d] = softmax_d(s[r,b,:])
`v` is never used.

Strategy (v3): the masked row-sum over l — the only heavy op — runs on the
TensorEngine.  Host-side (layout only): samples are length-sorted and
serpentine-dealt across the 8 cores so all cores share one compiled
module; each sample's first len[b] k-rows (fp16, padded to a 4-row
multiple) are packed back-to-back and interleaved even/odd into PAIRS of
128-row sub-slabs.  Because sample boundaries are even, both sub-slabs of
a pair share one [128, 32] 0/1 ownership mask (which packed row belongs
to which sample slot), so one matmul per pair reduces 256 k-rows:
stationary = the mask, moving = [128, 2*192] (N=384 hides the weight
load; PSUM bank holds exactly 384 fp32), accumulating each sample's
row-sum pair into its PSUM partition.  Slots live in two PSUM half-tiles
of 64 (matmul out base partitions limited to {0,32,64}); a final DVE fold
adds the even/odd halves, and the finish (3x3 mix, *qsum, softmax) runs
per half — half A's finish overlaps half B's matmuls.  First matmul per
32-slot page uses start=True so PSUM needs no zero-fill.  Masking and
ragged lengths are free: no per-block masks, no partial-row correction,
and k traffic drops 19.7MB -> ~5.0MB/core.  fp16 k gives rel_err ~1.3e-2
(<2e-2 gate, deterministic for the fixed-seed inputs); q must stay fp32
(fp16 q measured 1.85e-2) so qsum is a DVE reduce over host-transposed
(b, d, lq) q.  DMA: all k-pair chunks stay resident in SBUF (no slot
recycling to stall the ring) on the Sync HWDGE ring along with aux and
the output store (the ACT ring generates descriptors ~10x slower); masks
then q ride the SWDGE queue.
"""

import numpy as np

try:
    import concourse.bass as bass
except ImportError:  # pragma: no cover - path fallback
    import sys

    sys.path.insert(0, "/opt/trn_rl_repo")
    import concourse.bass as bass

import concourse.bacc as bacc
import concourse.mybir as mybir
import concourse.tile as tile
from concourse.bass_utils import run_bass_kernel_spmd

F32 = mybir.dt.float32
F16 = mybir.dt.float16

NCORES = 8
B = 1024
BL = B // NCORES  # 128 sample slots per core
HB = BL // 2  # 64 slots per PSUM half
LQ = 64
LK = 200
D = 64
KD = 3 * D  # 192
PAD = 4  # per-sample row padding granularity (must be even)
SLAB = 128  # rows per sub-slab = matmul contraction dim
PAIR = 2 * SLAB  # rows per slab pair = one matmul
PAGE = 32  # PSUM partition page (out base partition must be 0/32/64)

_CACHE = {}


def _plan(lens):
    """Global packing plan shared by all cores (uniform compiled module)."""
    order = np.argsort(-lens, kind="stable")
    slot_sample = np.empty((NCORES, BL), np.int64)
    for t in range(BL // 2):
        rk = order[16 * t : 16 * t + 16]
        for c in range(NCORES):
            slot_sample[c, 2 * t] = rk[c]
            slot_sample[c, 2 * t + 1] = rk[15 - c]
    slens = lens[slot_sample]  # (8, 128)
    plens = ((slens + PAD - 1) // PAD) * PAD
    starts = np.zeros((NCORES, BL + 1), np.int64)
    starts[:, 1:] = np.cumsum(plens, axis=1)
    T = int(-(-starts[:, -1].max() // PAIR))  # number of slab pairs
    mm = []
    for s in range(T):
        pages = set()
        lo, hi = PAIR * s, PAIR * (s + 1)
        for c in range(NCORES):
            a = int(np.searchsorted(starts[c, 1:], lo, side="right"))
            b_ = int(np.searchsorted(starts[c, :-1], hi, side="left"))
            for p in range(a, b_):
                if plens[c, p] > 0:
                    pages.add(p // PAGE)
        for pg in sorted(pages):
            mm.append((s, pg))
    have = {pg for _, pg in mm}
    for pg in range(BL // PAGE):
        if pg not in have:  # stale-PSUM guard: zero-mask matmul inits the page
            mm.append((max(T - 1, 0), pg))
    mm.sort()
    return slot_sample, slens, plens, starts, T, mm


def _chunks(T):
    """Pair-chunk sizes: small first chunk for an early compute start."""
    sizes = [3, 8]
    left = T - sum(sizes)
    while left > 0:
        r = min(15, left)
        sizes.append(r)
        left -= r
    return sizes


def _mm_flags(mm):
    first_of_page = [False] * len(mm)
    last_of_page = [False] * len(mm)
    seen = set()
    for i, (_s, pg) in enumerate(mm):
        if pg not in seen:
            seen.add(pg)
            first_of_page[i] = True
    seen = set()
    for i in range(len(mm) - 1, -1, -1):
        pg = mm[i][1]
        if pg not in seen:
            seen.add(pg)
            last_of_page[i] = True
    return first_of_page, last_of_page


def _build_module(T, mm):
    nc = bacc.Bacc("TRN2", target_bir_lowering=False, debug=False)
    n_mm = len(mm)
    first_of_page, last_of_page = _mm_flags(mm)

    # kslab[r, t, h, d] = packed row (256 t + 2 r + h), fp16
    k_d = nc.dram_tensor("kslab", [SLAB, T, 2, KD], F16, kind="ExternalInput").ap()
    m_d = nc.dram_tensor("masks", [SLAB, n_mm, PAGE], F16, kind="ExternalInput").ap()
    q_d = nc.dram_tensor("q", [BL, D, LQ], F32, kind="ExternalInput").ap()
    aux_d = nc.dram_tensor("aux", [HB, 9], F32, kind="ExternalInput").ap()
    out_d = nc.dram_tensor("out", [BL, KD], F32, kind="ExternalOutput").ap()

    mult = mybir.AluOpType.mult
    add = mybir.AluOpType.add
    AX = mybir.AxisListType.X

    chunks = _chunks(T)
    mm_by_chunk = []
    s0 = 0
    i0 = 0
    for R in chunks:
        i1 = i0
        while i1 < n_mm and mm[i1][0] < s0 + R:
            i1 += 1
        mm_by_chunk.append((s0, R, i0, i1))
        s0 += R
        i0 = i1
    assert i0 == n_mm

    with tile.TileContext(nc) as tc:
        with (
            tc.tile_pool(name="singles", bufs=1) as singles,
            tc.tile_pool(name="psum", bufs=1, space="PSUM") as psum_pool,
            tc.tile_pool(name="small", bufs=2) as small,
        ):
            aux_t = singles.tile([HB, 9], F32)
            nc.sync.dma_start(out=aux_t, in_=aux_d)

            # masks first on SWDGE (needed by the first matmul), then q
            mt = singles.tile([SLAB, n_mm, PAGE], F16)
            nc.gpsimd.dma_start(out=mt, in_=m_d)
            q_h = []
            for h in range(2):
                qt = singles.tile([HB, D, LQ], F32, tag=f"q{h}", name=f"q{h}")
                nc.gpsimd.dma_start(out=qt, in_=q_d[h * HB : (h + 1) * HB, :, :])
                q_h.append(qt)

            psum_h = [
                psum_pool.tile([HB, 2, KD], F32, tag=f"ps{h}", name=f"psum{h}")
                for h in range(2)
            ]

            # k pair chunks: all resident (no recycling), Sync HWDGE ring
            staged = []
            for ci, (s0, R, i0, i1) in enumerate(mm_by_chunk):
                kt = singles.tile([SLAB, R, 2, KD], F16, tag="kt", name=f"k{ci}")
                nc.sync.dma_start(out=kt, in_=k_d[:, s0 : s0 + R, :, :])
                staged.append((s0, R, i0, i1, kt))

            for s0, R, i0, i1, kt in staged:
                for i in range(i0, i1):
                    s, pg = mm[i]
                    ph = psum_h[pg // 2]
                    off = (pg % 2) * PAGE
                    nc.tensor.matmul(
                        ph[off : off + PAGE, :, :],
                        mt[:, i, :],
                        kt[:, s - s0, :, :],
                        start=first_of_page[i],
                        stop=last_of_page[i],
                        skip_group_check=True,
                    )

            # qsums first in DVE order: off the critical tail
            qs_h = []
            for h in range(2):
                qs = small.tile([HB, D], F32, tag=f"qs{h}", name=f"qs{h}")
                nc.vector.reduce_sum(out=qs[:, :], in_=q_h[h][:, :, :], axis=AX)
                qs_h.append(qs)

            obuf_h = []
            for h in range(2):
                psum_t = psum_h[h]
                qs = qs_h[h]
                ev = small.tile([HB, KD], F32, tag=f"ev{h}", name=f"ev{h}")
                nc.scalar.copy(out=ev[:, :], in_=psum_t[:, 1, :])
                ks = small.tile([HB, KD], F32, tag=f"ks{h}", name=f"ks{h}")
                nc.vector.tensor_tensor(
                    out=ks[:, :], in0=psum_t[:, 0, :], in1=ev[:, :], op=add
                )
                obuf = singles.tile([HB, KD], F32, tag=f"ob{h}", name=f"ob{h}")
                obuf_h.append(obuf)
                for r3 in range(3):
                    t1 = small.tile([HB, D], F32, tag="t1")
                    nc.vector.tensor_scalar(
                        out=t1[:, :],
                        in0=ks[:, 2 * D : 3 * D],
                        scalar1=aux_t[:, 3 * r3 + 2 : 3 * r3 + 3],
                        scalar2=None,
                        op0=mult,
                    )
                    t2 = small.tile([HB, D], F32, tag="t2")
                    nc.vector.scalar_tensor_tensor(
                        out=t2[:, :],
                        in0=ks[:, D : 2 * D],
                        scalar=aux_t[:, 3 * r3 + 1 : 3 * r3 + 2],
                        in1=t1[:, :],
                        op0=mult,
                        op1=add,
                    )
                    t3 = small.tile([HB, D], F32, tag="t3")
                    nc.vector.scalar_tensor_tensor(
                        out=t3[:, :],
                        in0=ks[:, 0:D],
                        scalar=aux_t[:, 3 * r3 : 3 * r3 + 1],
                        in1=t2[:, :],
                        op0=mult,
                        op1=add,
                    )
                    s_r = small.tile([HB, D], F32, tag="sr")
                    nc.vector.tensor_mul(out=s_r[:, :], in0=t3[:, :], in1=qs[:, :])
                    mx = small.tile([HB, 1], F32, tag="mx")
                    nc.vector.reduce_max(out=mx[:, :], in_=s_r[:, :], axis=AX)
                    nmx = small.tile([HB, 1], F32, tag="nmx")
                    nc.vector.tensor_scalar_mul(
                        out=nmx[:, :], in0=mx[:, :], scalar1=-1.0
                    )
                    ex = small.tile([HB, D], F32, tag="ex")
                    esum = small.tile([HB, 1], F32, tag="esum")
                    nc.scalar.activation(
                        out=ex[:, :],
                        in_=s_r[:, :],
                        func=mybir.ActivationFunctionType.Exp,
                        bias=nmx[:, :],
                        scale=1.0,
                        accum_out=esum[:, :],
                    )
                    rec = small.tile([HB, 1], F32, tag="rec")
                    nc.vector.reciprocal(out=rec[:, :], in_=esum[:, :])
                    nc.scalar.activation(
                        out=obuf[:, r3 * D : (r3 + 1) * D],
                        in_=ex[:, :],
                        func=mybir.ActivationFunctionType.Copy,
                        bias=0.0,
                        scale=rec[:, :],
                    )

            for h in range(2):
                nc.sync.dma_start(
                    out=out_d[h * HB : (h + 1) * HB, :], in_=obuf_h[h][:, :]
                )

    nc.compile()
    return nc


def _get_module(T, mm):
    key = (T, tuple(mm))
    nc = _CACHE.get(key)
    if nc is None:
        nc = _build_module(T, mm)
        _CACHE[key] = nc
    return nc


def _prepare(q, k16, W, plan):
    slot_sample, slens, plens, starts, T, mm = plan
    n_mm = len(mm)
    w_rep = np.tile(W.reshape(1, 9), (HB, 1)).astype(np.float32)
    in_maps = []
    for c in range(NCORES):
        rows = np.zeros((T * PAIR, KD), np.float16)
        for p in range(BL):
            L = int(slens[c, p])
            if L > 0:
                st = int(starts[c, p])
                rows[st : st + L] = k16[slot_sample[c, p], :L]
        # packed row g -> (pair t = g//256, sub-slab h = g%2, row r = (g%256)//2)
        kslab = np.ascontiguousarray(
            rows.reshape(T, SLAB, 2, KD).transpose(1, 0, 2, 3)
        )  # [128, T, 2, 192]

        masks = np.zeros((n_mm, SLAB, PAGE), np.float16)
        for i, (s, pg) in enumerate(mm):
            base = PAIR * s
            for p in range(pg * PAGE, (pg + 1) * PAGE):
                st, L = int(starts[c, p]), int(slens[c, p])
                # ownership at even-row granularity: rows st..st+L-1 are real;
                # in-pair position g-base maps to mask row (g-base)//2.  L may
                # be odd: the padded odd row is zero data, safe to include.
                lo = max(st, base)
                hi = min(st + int(plens[c, p]), base + PAIR)
                if hi > lo:
                    masks[i, (lo - base) // 2 : (hi - base) // 2, p - pg * PAGE] = 1.0
        maskst = np.ascontiguousarray(masks.transpose(1, 0, 2))  # [128, n_mm, 32]

        qt = np.ascontiguousarray(q[slot_sample[c]].transpose(0, 2, 1))
        in_maps.append({"kslab": kslab, "masks": maskst, "q": qt, "aux": w_rep})
    return in_maps


def _run(q, k, kes_length, mss_weight, **run_kwargs):
    q = np.ascontiguousarray(np.asarray(q, dtype=np.float32))
    k = np.asarray(k, dtype=np.float32)
    lens = np.asarray(kes_length).astype(np.int64).reshape(B)
    m = np.asarray(mss_weight, dtype=np.float32)
    e = np.exp(m - m.max(axis=1, keepdims=True))
    W = (e / e.sum(axis=1, keepdims=True)).astype(np.float32)

    plan = _plan(lens)
    slot_sample = plan[0]
    T, mm = plan[4], plan[5]
    nc = _get_module(T, mm)
    k16 = k.astype(np.float16)
    in_maps = _prepare(q, k16, W, plan)
    res = run_bass_kernel_spmd(nc, in_maps, core_ids=list(range(NCORES)), **run_kwargs)
    out = np.empty((B, KD), np.float32)
    for c in range(NCORES):
        out[slot_sample[c]] = res.results[c]["out"]
    return out.reshape(B, 1, KD), res


def kernel(q, k, v=None, kes_length=None, mss_weight=None, **_):
    out, _res = _run(q, k, kes_length, mss_weight)
    return out


# revision 9
# speedup vs baseline: 1.5250x; 1.5250x over previous
"""Trainium2 Bass kernel for nn_AutoAttention_Layer (sparse_attention).

Math (from the reference):
    W    = softmax(mss_weight, axis=1)                      # (3,3)
    qsum = sum_j q[b,j,:]                                   # (B,D)
    ksum_s[b,d] = sum_{l < len[b]} k[b,l,s*D+d]             # (B,3,D)
    s[r,b,d]    = (sum_s W[r,s]*ksum_s[b,d]) * qsum[b,d]
    out[b,0,r*D+d] = softmax_d(s[r,b,:])
`v` is never used.

Strategy (v3): the masked row-sum over l — the only heavy op — runs on the
TensorEngine.  Host-side (layout only): samples are length-sorted and
serpentine-dealt across the 8 cores so all cores share one compiled
module; each sample's first len[b] k-rows (fp16, padded to a 4-row
multiple) are packed back-to-back and interleaved even/odd into PAIRS of
128-row sub-slabs.  Because sample boundaries are even, both sub-slabs of
a pair share one [128, 32] 0/1 ownership mask (which packed row belongs
to which sample slot), so one matmul per pair reduces 256 k-rows:
stationary = the mask, moving = [128, 2*192] (N=384 hides the weight
load; PSUM bank holds exactly 384 fp32), accumulating each sample's
row-sum pair into its PSUM partition.  Slots live in two PSUM half-tiles
of 64 (matmul out base partitions limited to {0,32,64}); a final DVE fold
adds the even/odd halves, and the finish (3x3 mix, *qsum, softmax) runs
per half — half A's finish overlaps half B's matmuls.  First matmul per
32-slot page uses start=True so PSUM needs no zero-fill.  Masking and
ragged lengths are free: no per-block masks, no partial-row correction,
and k traffic drops 19.7MB -> ~5.0MB/core.  fp16 k gives rel_err ~1.3e-2
(<2e-2 gate, deterministic for the fixed-seed inputs); q must stay fp32
(fp16 q measured 1.85e-2) so qsum is a DVE reduce over host-transposed
(b, d, lq) q.  DMA: all k-pair chunks stay resident in SBUF (no slot
recycling to stall the ring) on the Sync HWDGE ring along with aux and
the output store (the ACT ring generates descriptors ~10x slower); masks
then q ride the SWDGE queue.
"""

import numpy as np

try:
    import concourse.bass as bass
except ImportError:  # pragma: no cover - path fallback
    import sys

    sys.path.insert(0, "/opt/trn_rl_repo")
    import concourse.bass as bass

import concourse.bacc as bacc
import concourse.mybir as mybir
import concourse.tile as tile
from concourse.bass_utils import run_bass_kernel_spmd

F32 = mybir.dt.float32
F16 = mybir.dt.float16

NCORES = 8
B = 1024
BL = B // NCORES  # 128 sample slots per core
HB = BL // 2  # 64 slots per PSUM half
LQ = 64
LK = 200
D = 64
KD = 3 * D  # 192
PAD = 4  # per-sample row padding granularity (must be even)
SLAB = 128  # rows per sub-slab = matmul contraction dim
PAIR = 2 * SLAB  # rows per slab pair = one matmul
PAGE = 32  # PSUM partition page (out base partition must be 0/32/64)

_CACHE = {}


def _plan(lens):
    """Global packing plan shared by all cores (uniform compiled module)."""
    order = np.argsort(-lens, kind="stable")
    slot_sample = np.empty((NCORES, BL), np.int64)
    for t in range(BL // 2):
        rk = order[16 * t : 16 * t + 16]
        for c in range(NCORES):
            slot_sample[c, 2 * t] = rk[c]
            slot_sample[c, 2 * t + 1] = rk[15 - c]
    slens = lens[slot_sample]  # (8, 128)
    plens = ((slens + PAD - 1) // PAD) * PAD
    starts = np.zeros((NCORES, BL + 1), np.int64)
    starts[:, 1:] = np.cumsum(plens, axis=1)
    T = int(-(-starts[:, -1].max() // PAIR))  # number of slab pairs
    mm = []
    for s in range(T):
        pages = set()
        lo, hi = PAIR * s, PAIR * (s + 1)
        for c in range(NCORES):
            a = int(np.searchsorted(starts[c, 1:], lo, side="right"))
            b_ = int(np.searchsorted(starts[c, :-1], hi, side="left"))
            for p in range(a, b_):
                if plens[c, p] > 0:
                    pages.add(p // PAGE)
        for pg in sorted(pages):
            mm.append((s, pg))
    have = {pg for _, pg in mm}
    for pg in range(BL // PAGE):
        if pg not in have:  # stale-PSUM guard: zero-mask matmul inits the page
            mm.append((max(T - 1, 0), pg))
    mm.sort()
    return slot_sample, slens, plens, starts, T, mm


def _chunks(T):
    """Pair-chunk sizes: small first chunk for an early compute start."""
    sizes = [3, 8]
    left = T - sum(sizes)
    while left > 0:
        r = min(15, left)
        sizes.append(r)
        left -= r
    return sizes


def _mm_flags(mm):
    first_of_page = [False] * len(mm)
    last_of_page = [False] * len(mm)
    seen = set()
    for i, (_s, pg) in enumerate(mm):
        if pg not in seen:
            seen.add(pg)
            first_of_page[i] = True
    seen = set()
    for i in range(len(mm) - 1, -1, -1):
        pg = mm[i][1]
        if pg not in seen:
            seen.add(pg)
            last_of_page[i] = True
    return first_of_page, last_of_page


def _build_module(T, mm):
    nc = bacc.Bacc("TRN2", target_bir_lowering=False, debug=False)
    n_mm = len(mm)
    first_of_page, last_of_page = _mm_flags(mm)

    # kslab[r, t, h, d] = packed row (256 t + 2 r + h), fp16
    k_d = nc.dram_tensor("kslab", [SLAB, T, 2, KD], F16, kind="ExternalInput").ap()
    m_d = nc.dram_tensor("masks", [SLAB, n_mm, PAGE], F16, kind="ExternalInput").ap()
    q_d = nc.dram_tensor("q", [BL, D, LQ], F32, kind="ExternalInput").ap()
    aux_d = nc.dram_tensor("aux", [HB, 9], F32, kind="ExternalInput").ap()
    out_d = nc.dram_tensor("out", [BL, KD], F32, kind="ExternalOutput").ap()

    mult = mybir.AluOpType.mult
    add = mybir.AluOpType.add
    AX = mybir.AxisListType.X

    chunks = _chunks(T)
    mm_by_chunk = []
    s0 = 0
    i0 = 0
    for R in chunks:
        i1 = i0
        while i1 < n_mm and mm[i1][0] < s0 + R:
            i1 += 1
        mm_by_chunk.append((s0, R, i0, i1))
        s0 += R
        i0 = i1
    assert i0 == n_mm

    with tile.TileContext(nc) as tc:
        with (
            tc.tile_pool(name="singles", bufs=1) as singles,
            tc.tile_pool(name="psum", bufs=1, space="PSUM") as psum_pool,
            tc.tile_pool(name="small", bufs=2) as small,
        ):
            aux_t = singles.tile([HB, 9], F32)
            nc.sync.dma_start(out=aux_t, in_=aux_d)

            # masks gate every matmul: first on the (fast) Sync ring
            mt = singles.tile([SLAB, n_mm, PAGE], F16)
            nc.sync.dma_start(out=mt, in_=m_d)
            q_h = []
            for h in range(2):
                qt = singles.tile([HB, D, LQ], F32, tag=f"q{h}", name=f"q{h}")
                nc.gpsimd.dma_start(out=qt, in_=q_d[h * HB : (h + 1) * HB, :, :])
                q_h.append(qt)

            psum_h = [
                psum_pool.tile([HB, 2, KD], F32, tag=f"ps{h}", name=f"psum{h}")
                for h in range(2)
            ]

            # k pair chunks: all resident with DISTINCT tags (a shared tag
            # aliases the buffers and serializes each chunk's DMA behind the
            # previous chunk's matmuls), Sync HWDGE ring
            staged = []
            for ci, (s0, R, i0, i1) in enumerate(mm_by_chunk):
                kt = singles.tile([SLAB, R, 2, KD], F16, tag=f"kt{ci}", name=f"k{ci}")
                nc.sync.dma_start(out=kt, in_=k_d[:, s0 : s0 + R, :, :])
                staged.append((s0, R, i0, i1, kt))

            for s0, R, i0, i1, kt in staged:
                for i in range(i0, i1):
                    s, pg = mm[i]
                    ph = psum_h[pg // 2]
                    off = (pg % 2) * PAGE
                    nc.tensor.matmul(
                        ph[off : off + PAGE, :, :],
                        mt[:, i, :],
                        kt[:, s - s0, :, :],
                        start=first_of_page[i],
                        stop=last_of_page[i],
                        skip_group_check=True,
                    )

            # qsums first in DVE order: off the critical tail
            qs_h = []
            for h in range(2):
                qs = small.tile([HB, D], F32, tag=f"qs{h}", name=f"qs{h}")
                nc.vector.reduce_sum(out=qs[:, :], in_=q_h[h][:, :, :], axis=AX)
                qs_h.append(qs)

            def bcast(ap, dim, n):
                """Insert a stride-0 dim of size n at position dim (after partitions)."""
                newap = list(ap.ap)
                newap.insert(dim, [0, n])
                return bass.AP(tensor=ap.tensor, offset=ap.offset, ap=newap)

            obuf_h = []
            for h in range(2):
                psum_t = psum_h[h]
                qs = qs_h[h]
                # pair fold (one PSUM operand per instruction allowed)
                ev = small.tile([HB, KD], F32, tag=f"ev{h}", name=f"ev{h}")
                nc.scalar.copy(out=ev[:, :], in_=psum_t[:, 1, :])
                ks = small.tile([HB, KD], F32, tag=f"ks{h}", name=f"ks{h}")
                nc.vector.tensor_tensor(
                    out=ks[:, :], in0=psum_t[:, 0, :], in1=ev[:, :], op=add
                )
                # fused 3x3 mix over all r at once: aux holds W^T rows
                # (aux[:, 3s+r] = W[r, s]); macc[*, r, d] = sum_s W[r,s]*ks[*, s, d]
                macc = small.tile([HB, 3, D], F32, tag=f"ma{h}", name=f"ma{h}")
                tmp = small.tile([HB, 3, D], F32, tag=f"tm{h}", name=f"tm{h}")
                for s3 in range(3):
                    ks_b = bcast(ks[:, s3 * D : (s3 + 1) * D], 1, 3)  # [HB,3,D]
                    w_b = bcast(aux_t[:, 3 * s3 : 3 * s3 + 3], 2, D)  # [HB,3,D]
                    dst = macc if s3 == 0 else tmp
                    nc.vector.tensor_tensor(
                        out=dst[:, :, :], in0=ks_b, in1=w_b, op=mult
                    )
                    if s3 > 0:
                        nc.vector.tensor_tensor(
                            out=macc[:, :, :],
                            in0=macc[:, :, :],
                            in1=tmp[:, :, :],
                            op=add,
                        )
                # s = macc * qsum (same qsum for each r)
                s_r = small.tile([HB, 3, D], F32, tag=f"sr{h}", name=f"sr{h}")
                nc.vector.tensor_tensor(
                    out=s_r[:, :, :], in0=macc[:, :, :], in1=bcast(qs[:, :], 1, 3),
                    op=mult,
                )
                mx = small.tile([HB, 3], F32, tag=f"mx{h}", name=f"mx{h}")
                nc.vector.reduce_max(out=mx[:, :], in_=s_r[:, :, :], axis=AX)
                sm = small.tile([HB, 3, D], F32, tag=f"sm{h}", name=f"sm{h}")
                nc.vector.tensor_tensor(
                    out=sm[:, :, :], in0=s_r[:, :, :], in1=bcast(mx[:, :], 2, D),
                    op=mybir.AluOpType.subtract,
                )
                ex = small.tile([HB, 3, D], F32, tag=f"ex{h}", name=f"ex{h}")
                nc.scalar.activation(
                    out=ex[:, :, :],
                    in_=sm[:, :, :],
                    func=mybir.ActivationFunctionType.Exp,
                    bias=0.0,
                    scale=1.0,
                )
                es = small.tile([HB, 3], F32, tag=f"es{h}", name=f"es{h}")
                nc.vector.reduce_sum(out=es[:, :], in_=ex[:, :, :], axis=AX)
                rec = small.tile([HB, 3], F32, tag=f"rc{h}", name=f"rc{h}")
                nc.vector.reciprocal(out=rec[:, :], in_=es[:, :])
                obuf = singles.tile([HB, KD], F32, tag=f"ob{h}", name=f"ob{h}")
                obuf_h.append(obuf)
                ob3 = obuf.rearrange("p (r d) -> p r d", d=D)
                nc.vector.tensor_tensor(
                    out=ob3[:, :, :], in0=ex[:, :, :], in1=bcast(rec[:, :], 2, D),
                    op=mult,
                )

            for h in range(2):
                nc.sync.dma_start(
                    out=out_d[h * HB : (h + 1) * HB, :], in_=obuf_h[h][:, :]
                )

    nc.compile()
    return nc


def _get_module(T, mm):
    key = (T, tuple(mm))
    nc = _CACHE.get(key)
    if nc is None:
        nc = _build_module(T, mm)
        _CACHE[key] = nc
    return nc


def _prepare(q, k16, W, plan):
    slot_sample, slens, plens, starts, T, mm = plan
    n_mm = len(mm)
    w_rep = np.tile(W.T.reshape(1, 9), (HB, 1)).astype(np.float32)  # aux[:,3s+r]=W[r,s]
    in_maps = []
    for c in range(NCORES):
        rows = np.zeros((T * PAIR, KD), np.float16)
        for p in range(BL):
            L = int(slens[c, p])
            if L > 0:
                st = int(starts[c, p])
                rows[st : st + L] = k16[slot_sample[c, p], :L]
        # packed row g -> (pair t = g//256, sub-slab h = g%2, row r = (g%256)//2)
        kslab = np.ascontiguousarray(
            rows.reshape(T, SLAB, 2, KD).transpose(1, 0, 2, 3)
        )  # [128, T, 2, 192]

        masks = np.zeros((n_mm, SLAB, PAGE), np.float16)
        for i, (s, pg) in enumerate(mm):
            base = PAIR * s
            for p in range(pg * PAGE, (pg + 1) * PAGE):
                st, L = int(starts[c, p]), int(slens[c, p])
                # ownership at even-row granularity: rows st..st+L-1 are real;
                # in-pair position g-base maps to mask row (g-base)//2.  L may
                # be odd: the padded odd row is zero data, safe to include.
                lo = max(st, base)
                hi = min(st + int(plens[c, p]), base + PAIR)
                if hi > lo:
                    masks[i, (lo - base) // 2 : (hi - base) // 2, p - pg * PAGE] = 1.0
        maskst = np.ascontiguousarray(masks.transpose(1, 0, 2))  # [128, n_mm, 32]

        qt = np.ascontiguousarray(q[slot_sample[c]].transpose(0, 2, 1))
        in_maps.append({"kslab": kslab, "masks": maskst, "q": qt, "aux": w_rep})
    return in_maps


def _run(q, k, kes_length, mss_weight, **run_kwargs):
    q = np.ascontiguousarray(np.asarray(q, dtype=np.float32))
    k = np.asarray(k, dtype=np.float32)
    lens = np.asarray(kes_length).astype(np.int64).reshape(B)
    m = np.asarray(mss_weight, dtype=np.float32)
    e = np.exp(m - m.max(axis=1, keepdims=True))
    W = (e / e.sum(axis=1, keepdims=True)).astype(np.float32)

    plan = _plan(lens)
    slot_sample = plan[0]
    T, mm = plan[4], plan[5]
    nc = _get_module(T, mm)
    k16 = k.astype(np.float16)
    in_maps = _prepare(q, k16, W, plan)
    res = run_bass_kernel_spmd(nc, in_maps, core_ids=list(range(NCORES)), **run_kwargs)
    out = np.empty((B, KD), np.float32)
    for c in range(NCORES):
        out[slot_sample[c]] = res.results[c]["out"]
    return out.reshape(B, 1, KD), res


def kernel(q, k, v=None, kes_length=None, mss_weight=None, **_):
    out, _res = _run(q, k, kes_length, mss_weight)
    return out


# revision 12
# speedup vs baseline: 1.6030x; 1.0511x over previous
"""Trainium2 Bass kernel for nn_AutoAttention_Layer (sparse_attention).

Math (from the reference):
    W    = softmax(mss_weight, axis=1)                      # (3,3)
    qsum = sum_j q[b,j,:]                                   # (B,D)
    ksum_s[b,d] = sum_{l < len[b]} k[b,l,s*D+d]             # (B,3,D)
    s[r,b,d]    = (sum_s W[r,s]*ksum_s[b,d]) * qsum[b,d]
    out[b,0,r*D+d] = softmax_d(s[r,b,:])
`v` is never used.

Strategy (v3): the masked row-sum over l — the only heavy op — runs on the
TensorEngine.  Host-side (layout only): samples are length-sorted and
serpentine-dealt across the 8 cores so all cores share one compiled
module; each sample's first len[b] k-rows (fp16, padded to a 4-row
multiple) are packed back-to-back and interleaved even/odd into PAIRS of
128-row sub-slabs.  Because sample boundaries are even, both sub-slabs of
a pair share one [128, 32] 0/1 ownership mask (which packed row belongs
to which sample slot), so one matmul per pair reduces 256 k-rows:
stationary = the mask, moving = [128, 2*192] (N=384 hides the weight
load; PSUM bank holds exactly 384 fp32), accumulating each sample's
row-sum pair into its PSUM partition.  Slots live in two PSUM half-tiles
of 64 (matmul out base partitions limited to {0,32,64}); a final DVE fold
adds the even/odd halves, and the finish (3x3 mix, *qsum, softmax) runs
per half — half A's finish overlaps half B's matmuls.  First matmul per
32-slot page uses start=True so PSUM needs no zero-fill.  Masking and
ragged lengths are free: no per-block masks, no partial-row correction,
and k traffic drops 19.7MB -> ~5.0MB/core.  fp16 k gives rel_err ~1.3e-2
(<2e-2 gate, deterministic for the fixed-seed inputs); q must stay fp32
(fp16 q measured 1.85e-2) so qsum is a DVE reduce over host-transposed
(b, d, lq) q.  DMA: all k-pair chunks stay resident in SBUF (no slot
recycling to stall the ring) on the Sync HWDGE ring along with aux and
the output store (the ACT ring generates descriptors ~10x slower); masks
then q ride the SWDGE queue.
"""

import numpy as np

try:
    import concourse.bass as bass
except ImportError:  # pragma: no cover - path fallback
    import sys

    sys.path.insert(0, "/opt/trn_rl_repo")
    import concourse.bass as bass

import concourse.bacc as bacc
import concourse.mybir as mybir
import concourse.tile as tile
from concourse.bass_utils import run_bass_kernel_spmd

F32 = mybir.dt.float32
F16 = mybir.dt.float16

NCORES = 8
B = 1024
BL = B // NCORES  # 128 sample slots per core
HB = BL // 2  # 64 slots per PSUM half
LQ = 64
LK = 200
D = 64
KD = 3 * D  # 192
PAD = 4  # per-sample row padding granularity (must be even)
SLAB = 128  # rows per sub-slab = matmul contraction dim
PAIR = 2 * SLAB  # rows per slab pair = one matmul
PAGE = 32  # PSUM partition page (out base partition must be 0/32/64)

_CACHE = {}


def _plan(lens):
    """Global packing plan shared by all cores (uniform compiled module)."""
    order = np.argsort(-lens, kind="stable")
    slot_sample = np.empty((NCORES, BL), np.int64)
    for t in range(BL // 2):
        rk = order[16 * t : 16 * t + 16]
        for c in range(NCORES):
            slot_sample[c, 2 * t] = rk[c]
            slot_sample[c, 2 * t + 1] = rk[15 - c]
    slens = lens[slot_sample]  # (8, 128)
    plens = ((slens + PAD - 1) // PAD) * PAD
    starts = np.zeros((NCORES, BL + 1), np.int64)
    starts[:, 1:] = np.cumsum(plens, axis=1)
    T = int(-(-starts[:, -1].max() // PAIR))  # number of slab pairs
    mm = []
    for s in range(T):
        pages = set()
        lo, hi = PAIR * s, PAIR * (s + 1)
        for c in range(NCORES):
            a = int(np.searchsorted(starts[c, 1:], lo, side="right"))
            b_ = int(np.searchsorted(starts[c, :-1], hi, side="left"))
            for p in range(a, b_):
                if plens[c, p] > 0:
                    pages.add(p // PAGE)
        for pg in sorted(pages):
            mm.append((s, pg))
    have = {pg for _, pg in mm}
    for pg in range(BL // PAGE):
        if pg not in have:  # stale-PSUM guard: zero-mask matmul inits the page
            mm.append((max(T - 1, 0), pg))
    mm.sort()
    return slot_sample, slens, plens, starts, T, mm


def _chunks(T):
    """Pair-chunk sizes: small first chunk for an early compute start, small
    tail so the last matmuls aren't waiting on a fat transfer."""
    sizes = [2, 4]
    left = T - sum(sizes)
    while left > 8:
        r = min(8, left - 4)
        sizes.append(r)
        left -= r
    while left > 0:
        r = min(4, left)
        sizes.append(r)
        left -= r
    return sizes


def _mm_flags(mm):
    first_of_page = [False] * len(mm)
    last_of_page = [False] * len(mm)
    seen = set()
    for i, (_s, pg) in enumerate(mm):
        if pg not in seen:
            seen.add(pg)
            first_of_page[i] = True
    seen = set()
    for i in range(len(mm) - 1, -1, -1):
        pg = mm[i][1]
        if pg not in seen:
            seen.add(pg)
            last_of_page[i] = True
    return first_of_page, last_of_page


def _build_module(T, mm):
    nc = bacc.Bacc("TRN2", target_bir_lowering=False, debug=False)
    n_mm = len(mm)
    first_of_page, last_of_page = _mm_flags(mm)

    # kslab[r, t, h, d] = packed row (256 t + 2 r + h), fp16
    k_d = nc.dram_tensor("kslab", [SLAB, T, 2, KD], F16, kind="ExternalInput").ap()
    m_d = nc.dram_tensor("masks", [SLAB, n_mm, PAGE], F16, kind="ExternalInput").ap()
    q_d = nc.dram_tensor("q", [BL, D, LQ], F32, kind="ExternalInput").ap()
    aux_d = nc.dram_tensor("aux", [HB, 9], F32, kind="ExternalInput").ap()
    out_d = nc.dram_tensor("out", [BL, KD], F32, kind="ExternalOutput").ap()

    mult = mybir.AluOpType.mult
    add = mybir.AluOpType.add
    AX = mybir.AxisListType.X

    chunks = _chunks(T)
    mm_by_chunk = []
    s0 = 0
    i0 = 0
    for R in chunks:
        i1 = i0
        while i1 < n_mm and mm[i1][0] < s0 + R:
            i1 += 1
        mm_by_chunk.append((s0, R, i0, i1))
        s0 += R
        i0 = i1
    assert i0 == n_mm

    with tile.TileContext(nc) as tc:
        with (
            tc.tile_pool(name="singles", bufs=1) as singles,
            tc.tile_pool(name="psum", bufs=1, space="PSUM") as psum_pool,
            tc.tile_pool(name="small", bufs=2) as small,
        ):
            aux_t = singles.tile([HB, 9], F32)
            nc.sync.dma_start(out=aux_t, in_=aux_d)

            # masks gate every matmul: chunk 0's masks first on the Sync ring,
            # the rest after chunk 0's k data so the PE can start ASAP
            i1_0 = mm_by_chunk[0][3]
            mt0 = singles.tile([SLAB, max(i1_0, 1), PAGE], F16)
            if i1_0 > 0:
                nc.sync.dma_start(out=mt0[:, 0:i1_0, :], in_=m_d[:, 0:i1_0, :])
            # q on the ACT HWDGE ring (fat 16KB lines; SWDGE's slow software
            # line generation was tying up the DMA engines mid-stream)
            q_h = []
            for h in range(2):
                qt = singles.tile([HB, D, LQ], F32, tag=f"q{h}", name=f"q{h}")
                nc.scalar.dma_start(out=qt, in_=q_d[h * HB : (h + 1) * HB, :, :])
                q_h.append(qt)

            psum_h = [
                psum_pool.tile([HB, 2, KD], F32, tag=f"ps{h}", name=f"psum{h}")
                for h in range(2)
            ]

            # k pair chunks: all resident with DISTINCT tags (a shared tag
            # aliases the buffers and serializes each chunk's DMA behind the
            # previous chunk's matmuls), Sync HWDGE ring
            staged = []
            mt1 = None
            for ci, (s0, R, i0, i1) in enumerate(mm_by_chunk):
                kt = singles.tile([SLAB, R, 2, KD], F16, tag=f"kt{ci}", name=f"k{ci}")
                nc.sync.dma_start(out=kt, in_=k_d[:, s0 : s0 + R, :, :])
                if ci == 0 and n_mm > i1_0:
                    mt1 = singles.tile([SLAB, n_mm - i1_0, PAGE], F16)
                    nc.sync.dma_start(out=mt1, in_=m_d[:, i1_0:n_mm, :])
                staged.append((s0, R, i0, i1, kt))

            for s0, R, i0, i1, kt in staged:
                for i in range(i0, i1):
                    s, pg = mm[i]
                    ph = psum_h[pg // 2]
                    off = (pg % 2) * PAGE
                    lhsT = mt0[:, i, :] if i < i1_0 else mt1[:, i - i1_0, :]
                    nc.tensor.matmul(
                        ph[off : off + PAGE, :, :],
                        lhsT,
                        kt[:, s - s0, :, :],
                        start=first_of_page[i],
                        stop=last_of_page[i],
                        skip_group_check=True,
                    )

            # qsums first in DVE order: off the critical tail
            qs_h = []
            for h in range(2):
                qs = small.tile([HB, D], F32, tag=f"qs{h}", name=f"qs{h}")
                nc.vector.reduce_sum(out=qs[:, :], in_=q_h[h][:, :, :], axis=AX)
                qs_h.append(qs)

            def bcast(ap, dim, n):
                """Insert a stride-0 dim of size n at position dim (after partitions)."""
                newap = list(ap.ap)
                newap.insert(dim, [0, n])
                return bass.AP(tensor=ap.tensor, offset=ap.offset, ap=newap)

            obuf_h = []
            for h in range(2):
                psum_t = psum_h[h]
                qs = qs_h[h]
                # pair fold (one PSUM operand per instruction allowed)
                ev = small.tile([HB, KD], F32, tag=f"ev{h}", name=f"ev{h}")
                nc.scalar.copy(out=ev[:, :], in_=psum_t[:, 1, :])
                ks = small.tile([HB, KD], F32, tag=f"ks{h}", name=f"ks{h}")
                nc.vector.tensor_tensor(
                    out=ks[:, :], in0=psum_t[:, 0, :], in1=ev[:, :], op=add
                )
                # fused 3x3 mix over all r at once: aux holds W^T rows
                # (aux[:, 3s+r] = W[r, s]); macc[*, r, d] = sum_s W[r,s]*ks[*, s, d]
                macc = small.tile([HB, 3, D], F32, tag=f"ma{h}", name=f"ma{h}")
                tmp = small.tile([HB, 3, D], F32, tag=f"tm{h}", name=f"tm{h}")
                for s3 in range(3):
                    ks_b = bcast(ks[:, s3 * D : (s3 + 1) * D], 1, 3)  # [HB,3,D]
                    w_b = bcast(aux_t[:, 3 * s3 : 3 * s3 + 3], 2, D)  # [HB,3,D]
                    dst = macc if s3 == 0 else tmp
                    nc.vector.tensor_tensor(
                        out=dst[:, :, :], in0=ks_b, in1=w_b, op=mult
                    )
                    if s3 > 0:
                        nc.vector.tensor_tensor(
                            out=macc[:, :, :],
                            in0=macc[:, :, :],
                            in1=tmp[:, :, :],
                            op=add,
                        )
                # s = macc * qsum (same qsum for each r)
                s_r = small.tile([HB, 3, D], F32, tag=f"sr{h}", name=f"sr{h}")
                nc.vector.tensor_tensor(
                    out=s_r[:, :, :], in0=macc[:, :, :], in1=bcast(qs[:, :], 1, 3),
                    op=mult,
                )
                mx = small.tile([HB, 3], F32, tag=f"mx{h}", name=f"mx{h}")
                nc.vector.reduce_max(out=mx[:, :], in_=s_r[:, :, :], axis=AX)
                sm = small.tile([HB, 3, D], F32, tag=f"sm{h}", name=f"sm{h}")
                nc.vector.tensor_tensor(
                    out=sm[:, :, :], in0=s_r[:, :, :], in1=bcast(mx[:, :], 2, D),
                    op=mybir.AluOpType.subtract,
                )
                ex = small.tile([HB, 3, D], F32, tag=f"ex{h}", name=f"ex{h}")
                nc.scalar.activation(
                    out=ex[:, :, :],
                    in_=sm[:, :, :],
                    func=mybir.ActivationFunctionType.Exp,
                    bias=0.0,
                    scale=1.0,
                )
                es = small.tile([HB, 3], F32, tag=f"es{h}", name=f"es{h}")
                nc.vector.reduce_sum(out=es[:, :], in_=ex[:, :, :], axis=AX)
                rec = small.tile([HB, 3], F32, tag=f"rc{h}", name=f"rc{h}")
                nc.vector.reciprocal(out=rec[:, :], in_=es[:, :])
                obuf = singles.tile([HB, KD], F32, tag=f"ob{h}", name=f"ob{h}")
                obuf_h.append(obuf)
                ob3 = obuf.rearrange("p (r d) -> p r d", d=D)
                nc.vector.tensor_tensor(
                    out=ob3[:, :, :], in0=ex[:, :, :], in1=bcast(rec[:, :], 2, D),
                    op=mult,
                )

            for h in range(2):
                nc.sync.dma_start(
                    out=out_d[h * HB : (h + 1) * HB, :], in_=obuf_h[h][:, :]
                )

    nc.compile()
    return nc


def _get_module(T, mm):
    key = (T, tuple(mm))
    nc = _CACHE.get(key)
    if nc is None:
        nc = _build_module(T, mm)
        _CACHE[key] = nc
    return nc


def _prepare(q, k16, W, plan):
    slot_sample, slens, plens, starts, T, mm = plan
    n_mm = len(mm)
    w_rep = np.tile(W.T.reshape(1, 9), (HB, 1)).astype(np.float32)  # aux[:,3s+r]=W[r,s]
    in_maps = []
    for c in range(NCORES):
        rows = np.zeros((T * PAIR, KD), np.float16)
        for p in range(BL):
            L = int(slens[c, p])
            if L > 0:
                st = int(starts[c, p])
                rows[st : st + L] = k16[slot_sample[c, p], :L]
        # packed row g -> (pair t = g//256, sub-slab h = g%2, row r = (g%256)//2)
        kslab = np.ascontiguousarray(
            rows.reshape(T, SLAB, 2, KD).transpose(1, 0, 2, 3)
        )  # [128, T, 2, 192]

        masks = np.zeros((n_mm, SLAB, PAGE), np.float16)
        for i, (s, pg) in enumerate(mm):
            base = PAIR * s
            for p in range(pg * PAGE, (pg + 1) * PAGE):
                st, L = int(starts[c, p]), int(slens[c, p])
                # ownership at even-row granularity: rows st..st+L-1 are real;
                # in-pair position g-base maps to mask row (g-base)//2.  L may
                # be odd: the padded odd row is zero data, safe to include.
                lo = max(st, base)
                hi = min(st + int(plens[c, p]), base + PAIR)
                if hi > lo:
                    masks[i, (lo - base) // 2 : (hi - base) // 2, p - pg * PAGE] = 1.0
        maskst = np.ascontiguousarray(masks.transpose(1, 0, 2))  # [128, n_mm, 32]

        qt = np.ascontiguousarray(q[slot_sample[c]].transpose(0, 2, 1))
        in_maps.append({"kslab": kslab, "masks": maskst, "q": qt, "aux": w_rep})
    return in_maps


def _run(q, k, kes_length, mss_weight, **run_kwargs):
    q = np.ascontiguousarray(np.asarray(q, dtype=np.float32))
    k = np.asarray(k, dtype=np.float32)
    lens = np.asarray(kes_length).astype(np.int64).reshape(B)
    m = np.asarray(mss_weight, dtype=np.float32)
    e = np.exp(m - m.max(axis=1, keepdims=True))
    W = (e / e.sum(axis=1, keepdims=True)).astype(np.float32)

    plan = _plan(lens)
    slot_sample = plan[0]
    T, mm = plan[4], plan[5]
    nc = _get_module(T, mm)
    k16 = k.astype(np.float16)
    in_maps = _prepare(q, k16, W, plan)
    res = run_bass_kernel_spmd(nc, in_maps, core_ids=list(range(NCORES)), **run_kwargs)
    out = np.empty((B, KD), np.float32)
    for c in range(NCORES):
        out[slot_sample[c]] = res.results[c]["out"]
    return out.reshape(B, 1, KD), res


def kernel(q, k, v=None, kes_length=None, mss_weight=None, **_):
    out, _res = _run(q, k, kes_length, mss_weight)
    return out
